# revision 58
# baseline (speedup 1.0000x reference)
"""Multi-head attention + output projection on 8 Trainium2 NeuronCores.

Problem (hardcoded): B=4, N=M=2048, D_IN=D_OUT=512, H=8, HD=VD=64.
  out = softmax(q @ k^T / sqrt(64)) @ v, heads concat, @ W_out.T + b_out

Sharding: each core owns (batch b = core//2, query-row half = core%2):
  q-chunk [1024, 512], full k/v for that batch, full W_out. All 8 heads are
  computed locally, so no collectives are needed; the host concatenates the
  8 disjoint [1024, 512] output chunks.

Device algorithm per core (S^T layout so the softmax reduction and the PV
contraction both land on the partition axis via matmuls; all matmuls in
float32r, which streams at 1 row/cycle for N>=256 — 4x faster than plain
fp32 and ~16x more accurate than bf16):
  per head-pair (2t, 2t+1) — the two heads sit at partition bases 0/64 of
  the same q^T/k^T o-tile, so their K=64 QK matmuls occupy disjoint PE
  row-groups and can run concurrently:
    S_T[j,i] = k_h^T q_h              (K=64, PSUM tiles s0/s1)
    P_T = exp(0.125*S_T)              (ScalarE — the bottleneck engine,
                                       ~134us busy of ~178us total)
    O_aug[65,i] += [v_h | 1]^T @ P_T  over 16 j-tiles (rows 0-63 = O_T
                                       unnormalized, row 64 = sumexp)
    normalize: copy O_aug to SBUF (frees the PSUM bank), DVE reciprocal of
    the sumexp row, DMA partition-move to row 0 (gpsimd partition_broadcast
    ucode reads the tile's partition 0, ignoring AP base), broadcast to 64
    rows, multiply -> normalized O^T packed [even head rows 0-63 | odd head
    rows 64-127] per pair for the projection.
  projection: per 128-row i-chunk, one K=128 matmul per head pair (pairs
  0-2; pair 3 unfused from its staging tiles so no DMA sits on the tail)
  plus a K=1 ones matmul adding b_out.
  Tail minimization: the last pair runs its two heads sequentially (each
  double-buffered across the s0/s1 PSUM slots) so head 6's normalization
  hides under head 7's pass; head 7's normalization is interleaved with
  the projection in i-quarters, its recip broadcast done by a K=1 matmul
  into the freed o0 PSUM slot. Startup: only qt0 rides the ScalarE HWDGE
  ring (anything more would queue ahead of the first exp in the ACT
  sequencer FIFO); loads are ordered by first-use time (kt0 head chunk,
  qt0 halves, va0 whole, later groups, later pairs' k/q). The bias rides
  as a 65th contraction row of pair-3's even-head projection matmul
  (ones row parked in the unused ot_sb[64, 3] partitions, [W6; b] rhs).
  Cost-model exec ~165 us/core (ScalarE-exp busy ~134 us is the floor set
  by PSUM geometry: wider exp tiles would need >8 banks); HW rel err vs
  the fp32 reference ~2.6e-4.
"""

import numpy as np

B, N, M, D, H, HD = 4, 2048, 2048, 512, 8, 64
NLOC = N // 2  # query rows per core
NCORES = 8
VA_C = 66  # per-head packed v columns: 64 v + 1 ones + 1 pad


def _build_bass(debug=False):
    import concourse.mybir as mybir
    import concourse.tile as tile
    from concourse import bacc

    f32 = mybir.dt.float32
    f32r = mybir.dt.float32r

    nc = bacc.Bacc()
    qt_d = nc.dram_tensor("qt", [D, NLOC], f32r, kind="ExternalInput")
    kt_d = nc.dram_tensor("kt", [D, M], f32r, kind="ExternalInput")
    va_d = nc.dram_tensor("va", [M, H, VA_C], f32r, kind="ExternalInput")
    wt_d = nc.dram_tensor("wt", [128, H // 2, D], f32r, kind="ExternalInput")
    wt7_d = nc.dram_tensor("wt7", [HD, D], f32r, kind="ExternalInput")
    on_d = nc.dram_tensor("on1", [1, NLOC], f32r, kind="ExternalInput")
    wt6b_d = nc.dram_tensor("wt6b", [HD + 1, D], f32r, kind="ExternalInput")
    out_d = nc.dram_tensor("out", [NLOC, D], f32, kind="ExternalOutput")
    if debug:
        dbg = {
            "dbg_s": nc.dram_tensor("dbg_s", [128, NLOC], f32, kind="ExternalOutput"),
            "dbg_p": nc.dram_tensor("dbg_p", [128, NLOC], f32, kind="ExternalOutput"),
            "dbg_o": nc.dram_tensor("dbg_o", [HD + 1, NLOC], f32, kind="ExternalOutput"),
            "dbg_rc": nc.dram_tensor("dbg_rc", [1, NLOC], f32, kind="ExternalOutput"),
            "dbg_bc": nc.dram_tensor("dbg_bc", [HD, NLOC], f32, kind="ExternalOutput"),
            "dbg_ot": nc.dram_tensor("dbg_ot", [HD + 1, NLOC], f32, kind="ExternalOutput"),
        }

    JT = M // 128  # 16 j-tiles
    IC = NLOC // 512  # 2 i-chunks for matmul free dim

    with tile.TileContext(nc) as tc:
        with (
            tc.tile_pool(name="persist", bufs=1) as persist,
            tc.tile_pool(name="pt", bufs=3) as ptp,
            tc.tile_pool(name="work", bufs=2) as work,
            tc.tile_pool(name="ps_s", bufs=1, space="PSUM") as ps_s_pool,
            tc.tile_pool(name="ps_o", bufs=1, space="PSUM") as ps_o_pool,
        ):
            # split per o-tile so head-pair 0 can start before all loads land
            qt_sb = [persist.tile([128, NLOC], f32r, tag=f"qt{o}", name=f"qt{o}") for o in range(4)]
            kt_sb = [persist.tile([128, M], f32r, tag=f"kt{o}", name=f"kt{o}") for o in range(4)]
            qt_r = qt_d.rearrange("(o p) i -> o p i", p=128)
            kt_r = kt_d.rearrange("(o p) j -> o p j", p=128)
            va_r = va_d.rearrange("(g t p) h c -> g p t h c", p=128, g=4)
            va_sb = [persist.tile([128, JT // 4, H, VA_C], f32r, tag=f"va{g}", name=f"va{g}") for g in range(4)]
            nc.sync.dma_start(kt_sb[0][:, 0:128], kt_r[0, :, 0:128])
            nc.scalar.dma_start(qt_sb[0][:, 0:512], qt_r[0, :, 0:512])
            nc.scalar.dma_start(qt_sb[0][:, 512:1024], qt_r[0, :, 512:1024])
            nc.sync.dma_start(kt_sb[0][:, 128:512], kt_r[0, :, 128:512])
            nc.sync.dma_start(va_sb[0][:, 0:1], va_r[0, :, 0:1])
            nc.sync.dma_start(va_sb[0][:, 1:4], va_r[0, :, 1:4])
            nc.sync.dma_start(kt_sb[0][:, 512:1024], kt_r[0, :, 512:1024])
            nc.sync.dma_start(kt_sb[0][:, 1024:2048], kt_r[0, :, 1024:2048])
            # v groups cover j-tiles for ALL pairs: pair 0 streams through all
            # four within its first 33us, so they go before later pairs' q/k
            for g in range(1, 4):
                nc.sync.dma_start(va_sb[g], va_r[g])
            for o in range(1, 4):
                nc.sync.dma_start(kt_sb[o], kt_r[o])
                nc.sync.dma_start(qt_sb[o], qt_r[o])
            wt_sb = persist.tile([128, H // 2, D], f32r)
            nc.sync.dma_start(wt_sb, wt_d[:])
            wt7_sb = persist.tile([HD, D], f32r)
            nc.sync.dma_start(wt7_sb, wt7_d[:])
            wt6b_sb = persist.tile([HD + 1, D], f32r)
            nc.sync.dma_start(wt6b_sb, wt6b_d[:])
            ones65 = persist.tile([HD + 1, 128], f32)
            nc.sync.dma_start(ones65[HD : HD + 1, :], on_d[0:1, 0:128].bitcast(f32))

            # normalized O^T packed per head-pair: rows 0-63 even head,
            # rows 64-127 odd head (moved there by a partition-shift DMA) so
            # the projection contracts both heads in one K=128 matmul.
            ot_sb = persist.tile([128, H // 2, NLOC], f32r)
            nc.sync.dma_start(ot_sb[HD : HD + 1, 3, :], on_d[:])
            stg7 = persist.tile([HD, NLOC], f32r)

            def va_lhs(jt, h):
                return va_sb[jt // 4][:, jt % 4, h, 0 : HD + 1]

            def normalize(h, ps_o, cols=slice(0, NLOC)):
                # softmax normalization: O_T = O_unnorm / sumexp.
                # Copy PSUM->SBUF first so the accumulator bank frees early.
                n_c = cols.stop - cols.start
                oc = work.tile([HD + 1, NLOC], f32, tag=f"oc{h % 2}")
                nc.vector.tensor_copy(oc[:, 0:n_c], ps_o[0 : HD + 1, cols])
                rc = work.tile([HD + 1, NLOC], f32, tag="recip")
                nc.vector.reciprocal(rc[HD : HD + 1, 0:n_c], oc[HD : HD + 1, 0:n_c])
                # partition_broadcast's ucode reads the tile's partition 0,
                # ignoring the AP base — move the recip row there first.
                rc0 = work.tile([1, NLOC], f32, tag="recip0")
                nc.sync.dma_start(rc0[:, 0:n_c], rc[HD : HD + 1, 0:n_c])
                bc = work.tile([HD, NLOC], f32, tag="bcast")
                nc.gpsimd.partition_broadcast(bc[:, 0:n_c], rc0[:, 0:n_c])
                hp = h // 2
                if h % 2 == 0:
                    nc.vector.tensor_tensor(
                        ot_sb[0:HD, hp, cols],
                        oc[0:HD, 0:n_c],
                        bc[:, 0:n_c],
                        mybir.AluOpType.mult,
                    )
                elif h == 7:
                    nc.vector.tensor_tensor(
                        stg7[:, cols],
                        oc[0:HD, 0:n_c],
                        bc[:, 0:n_c],
                        mybir.AluOpType.mult,
                    )
                else:
                    stg = work.tile([HD, NLOC], f32r, tag="stg")
                    nc.vector.tensor_tensor(
                        stg[:, 0:n_c],
                        oc[0:HD, 0:n_c],
                        bc[:, 0:n_c],
                        mybir.AluOpType.mult,
                    )
                    nc.sync.dma_start(ot_sb[HD:128, hp, cols], stg[:, 0:n_c])
                if debug and h == 1:
                    nc.sync.dma_start(dbg["dbg_o"][:], oc)
                    nc.sync.dma_start(dbg["dbg_rc"][:], rc[HD : HD + 1, :])
                    nc.sync.dma_start(dbg["dbg_bc"][:], bc)
                    nc.sync.dma_start(dbg["dbg_ot"][0:HD, :], ot_sb[0:HD, 0, :].bitcast(f32))

            # output projection at the end: pairs 0-2 contract fused (K=128,
            # odd head staged into rows 64-127 during the run); pair 3 is
            # unfused (K=64 x2, head 7 read from its staging tile directly)
            # so no staging DMA sits on the critical tail. Bias via K=1 ones.
            def proj_chunks(icc_range):
              for icc in icc_range:
                csl = slice(icc * 128, (icc + 1) * 128)
                ps_f = ps_s_pool.tile([128, D], f32, tag=f"s{icc % 2}", name="ps_f")
                for hp in range(3):
                    nc.tensor.matmul(
                        ps_f,
                        lhsT=(ot_sb[:, hp, csl]),
                        rhs=(wt_sb[:, hp, :]),
                        start=(hp == 0),
                        stop=False,
                    )
                nc.tensor.matmul(
                    ps_f, lhsT=(ot_sb[0 : HD + 1, 3, csl]), rhs=(wt6b_sb[:]),
                    start=False, stop=False,
                )
                nc.tensor.matmul(
                    ps_f, lhsT=(stg7[:, csl]), rhs=(wt7_sb[:]),
                    start=False, stop=True,
                )
                f_sb = ptp.tile([128, D], f32, tag="fin")
                if icc % 2 == 0:
                    nc.vector.tensor_copy(f_sb, ps_f)
                else:
                    nc.scalar.copy(f_sb, ps_f)
                nc.sync.dma_start(out_d[csl, :], f_sb)

            # head pairs (2t, 2t+1) sit at partition bases 0/64 of o-tile t:
            # their QK matmuls use disjoint PE row-groups and run concurrently.
            def tail_pair():
                # pair 3: run the two heads sequentially, each double-buffered
                # across the s0/s1 slots, so head 6's normalization overlaps
                # head 7's whole pass and only head 7's chain is on the tail.
                ps_o0 = ps_o_pool.tile([HD + 1, NLOC], f32, tag="o0")
                ps_o1 = ps_o_pool.tile([HD + 1, NLOC], f32, tag="o1")
                for h, ps_o in ((6, ps_o0), (7, ps_o1)):
                    db = (h % 2) * HD
                    for jt in range(JT):
                        s = ps_s_pool.tile(
                            [128, NLOC], f32, tag=f"s{jt % 2}", name="s"
                        )
                        jsl = slice(jt * 128, (jt + 1) * 128)
                        for ic in range(IC):
                            isl = slice(ic * 512, (ic + 1) * 512)
                            nc.tensor.matmul(
                                s[:, isl],
                                lhsT=kt_sb[3][db : db + HD, jsl],
                                rhs=qt_sb[3][db : db + HD, isl],
                                start=True,
                                stop=True,
                            )
                        p = ptp.tile([128, NLOC], f32r, tag=f"p{jt % 2}", name="p")
                        nc.scalar.activation(
                            p, s, mybir.ActivationFunctionType.Exp, scale=0.125
                        )
                        for ic in range(IC):
                            isl = slice(ic * 512, (ic + 1) * 512)
                            nc.tensor.matmul(
                                ps_o[:, isl],
                                lhsT=va_lhs(jt, h),
                                rhs=p[:, isl],
                                start=(jt == 0),
                                stop=(jt == JT - 1),
                            )
                    if h == 6:
                        normalize(6, ps_o0)
                for q in range(4):
                    csl2 = slice(q * 256, (q + 1) * 256)
                    # head 7 quarter-normalize with the recip broadcast done
                    # by a K=1 matmul into the free o0 PSUM slot (no DMA
                    # partition-move / gpsimd op on the critical tail)
                    rc = work.tile([HD + 1, NLOC], f32, tag="recip", name="rc")
                    nc.vector.reciprocal(
                        rc[HD : HD + 1, 0:256], ps_o1[HD : HD + 1, csl2]
                    )
                    oc = work.tile([HD + 1, NLOC], f32, tag="oc1", name="oc")
                    nc.vector.tensor_copy(oc[:, 0:256], ps_o1[0 : HD + 1, csl2])
                    bc_ps = ps_o_pool.tile([HD, 256], f32, tag="o0", name="bc_ps")
                    nc.tensor.matmul(
                        bc_ps,
                        lhsT=ones65[HD : HD + 1, 0:HD],
                        rhs=rc[HD : HD + 1, 0:256],
                        start=True,
                        stop=True,
                    )
                    nc.vector.tensor_tensor(
                        stg7[:, csl2],
                        oc[0:HD, 0:256],
                        bc_ps,
                        mybir.AluOpType.mult,
                    )
                    proj_chunks(range(q * 2, (q + 1) * 2))

            for hp in range(H // 2 - 1):
                h0, h1 = 2 * hp, 2 * hp + 1
                ps_o0 = ps_o_pool.tile([HD + 1, NLOC], f32, tag="o0")
                ps_o1 = ps_o_pool.tile([HD + 1, NLOC], f32, tag="o1")
                for jt in range(JT):
                    s0 = ps_s_pool.tile([128, NLOC], f32, tag="s0")
                    s1 = ps_s_pool.tile([128, NLOC], f32, tag="s1")
                    jsl = slice(jt * 128, (jt + 1) * 128)
                    for ic in range(IC):
                        isl = slice(ic * 512, (ic + 1) * 512)
                        nc.tensor.matmul(
                            s0[:, isl],
                            lhsT=(kt_sb[hp][0:HD, jsl]),
                            rhs=(qt_sb[hp][0:HD, isl]),
                            start=True,
                            stop=True,
                        )
                        nc.tensor.matmul(
                            s1[:, isl],
                            lhsT=(kt_sb[hp][HD:128, jsl]),
                            rhs=(qt_sb[hp][HD:128, isl]),
                            start=True,
                            stop=True,
                        )
                    p0 = ptp.tile([128, NLOC], f32r, tag="p0")
                    nc.scalar.activation(
                        p0, s0, mybir.ActivationFunctionType.Exp, scale=0.125
                    )
                    p1 = ptp.tile([128, NLOC], f32r, tag="p1")
                    nc.scalar.activation(
                        p1, s1, mybir.ActivationFunctionType.Exp, scale=0.125
                    )
                    if debug and hp == 0 and jt == 0:
                        stg_s = work.tile([128, NLOC], f32, tag="dbg")
                        nc.vector.tensor_copy(stg_s, s1)
                        nc.sync.dma_start(dbg["dbg_s"][:], stg_s)
                        nc.sync.dma_start(dbg["dbg_p"][:], p1[:].bitcast(f32))
                    for ic in range(IC):
                        isl = slice(ic * 512, (ic + 1) * 512)
                        nc.tensor.matmul(
                            ps_o0[:, isl],
                            lhsT=va_lhs(jt, h0),
                            rhs=p0[:, isl],
                            start=(jt == 0),
                            stop=(jt == JT - 1),
                        )
                        nc.tensor.matmul(
                            ps_o1[:, isl],
                            lhsT=va_lhs(jt, h1),
                            rhs=p1[:, isl],
                            start=(jt == 0),
                            stop=(jt == JT - 1),
                        )
                normalize(h1, ps_o1)
                normalize(h0, ps_o0)
            tail_pair()

    nc.finalize()
    return nc


def _host_prep(q, k, v, W_out, b_out):
    """Shard + lay out inputs per core (pure layout: transpose/pack)."""
    q = np.asarray(q, dtype=np.float32)
    k = np.asarray(k, dtype=np.float32)
    v = np.asarray(v, dtype=np.float32)
    W_out = np.asarray(W_out, dtype=np.float32)
    b_out = np.asarray(b_out, dtype=np.float32)

    qT = np.ascontiguousarray(q.transpose(0, 2, 1))  # [B, D, N]
    kT = np.ascontiguousarray(k.transpose(0, 2, 1))  # [B, D, M]

    va = np.zeros((B, M, H, VA_C), dtype=np.float32)
    va[..., :HD] = v.reshape(B, M, H, HD)
    va[..., HD] = 1.0

    # wt[j2, hp, e] = W_out[e, hp*128 + j2] (two heads per 128-row block)
    wt = np.ascontiguousarray(W_out.T.reshape(H // 2, 128, D).transpose(1, 0, 2))

    in_maps = []
    for c in range(NCORES):
        b_, ih = divmod(c, 2)
        in_maps.append(
            {
                "qt": np.ascontiguousarray(qT[b_, :, ih * NLOC : (ih + 1) * NLOC]),
                "kt": kT[b_],
                "va": va[b_],
                "wt": wt,
                "wt7": np.ascontiguousarray(W_out.T[448:512, :]),
                "on1": np.ones((1, NLOC), np.float32),
                "wt6b": np.ascontiguousarray(
                    np.concatenate([W_out.T[384:448, :], b_out[None, :]], axis=0)
                ),
            }
        )
    return in_maps


def kernel(q, k, v, W_out, b_out):
    from concourse.bass_utils import run_bass_kernel_spmd

    nc = _build_bass()
    in_maps = _host_prep(q, k, v, W_out, b_out)
    res = run_bass_kernel_spmd(nc, in_maps, core_ids=list(range(NCORES)))
    out = np.empty((B, N, D), dtype=np.float32)
    for c, r_ in enumerate(res.results):
        b_, ih = divmod(c, 2)
        out[b_, ih * NLOC : (ih + 1) * NLOC, :] = r_["out"]
    return out
